# revision 38
# baseline (speedup 1.0000x reference)
"""GAT (2-layer, PyG-style) distributed Bass kernel for 8 Trainium2 NeuronCores.

Strategy (graph/data parallel, per sharding hint):
  - Nodes are partitioned into 8 contiguous blocks; core c owns destination
    nodes [c*N/8, (c+1)*N/8) and all edges incident to them (plus self loops).
  - Each layer: every core builds the full node feature table
    tbl[n] = [xh(n) | e_src(n)] in bf16 (redundant compute beats cross-core
    collectives), then processes its destination tiles: one hardware
    dma_gather per (tile-group x src-quarter) fetches per-edge source rows,
    attention coefficients are formed with Lrelu+Exp on the scalar engine
    (no max-subtraction needed: scores are O(1) so exp never overflows;
    softmax is exactly equivalent), and a 0/1 selection-matrix matmul in
    bf16 on the tensor engine performs the per-destination segment
    reduction of [msg | ea] in PSUM.
  - Host reassembles the transposed hidden table h_T from the 8 shards
    (pure data movement), then launch 2 repeats the same structure with
    the 40-wide single-head output layer.

All tensor-engine work is bf16 (1 cycle/row vs 4 for fp32); accumulation
stays fp32 in PSUM. Vector-engine work is batched per destination tile
(one is_equal builds all selection matrices of a tile; alpha/exp/msg-mult
are strided batch ops) to amortize per-instruction overheads.

SPMD constraints force fully uniform static structure across cores: every
(dst-tile x src-quarter) edge segment is padded to S chunks of 128 edges
(pad edges gather row 0 and use an out-of-range dst slot so selection
matrices zero them out). Source indices are split into 4 quarters because
dma_gather indices are int16.
"""

import math
import os
import sys

for _p in ("/opt/trn_rl_repo", "/root/.axon_site/_ro/trn_rl_repo"):
    if os.path.isdir(_p) and _p not in sys.path:
        sys.path.insert(0, _p)

import numpy as np
import ml_dtypes
from contextlib import ExitStack

import concourse.bacc as bacc
import concourse.bass as bass
import concourse.tile as tile
from concourse import mybir
from concourse.bass_utils import run_bass_kernel_spmd

F32 = mybir.dt.float32
BF16 = mybir.dt.bfloat16
I16 = mybir.dt.int16
AF = mybir.ActivationFunctionType
ALU = mybir.AluOpType

NEG_SLOPE = 0.2
EPS = 1e-16
P = 128
PAD_DST = 200.0  # sentinel dst_local for pad edges; never matches iota 0..127
BARRIER_EVERY = 10
USE_LRELU = False   # ACT Lrelu mis-lowers on HW (rel err 6e-2); keep DVE path
MAXGC = 7           # max gather size in 128-idx chunk columns per instruction
                    # (1024 idxs/gather verified on HW; 2048 wedges the core)


# --------------------------------------------------------------------------
# host-side graph preprocessing
# --------------------------------------------------------------------------

def _round_up(a, b):
    return (a + b - 1) // b * b


class EdgeStruct:
    """Uniform SPMD edge layout shared by both layers."""

    def __init__(self, src, dst, N, n_cores, G=5):
        self.N = N
        self.n_cores = n_cores
        self.G = G
        self.Npad = _round_up(N, 512)
        self.Qsz = self.Npad // 4
        assert self.Qsz <= 32767
        assert N % n_cores == 0
        self.npc = N // n_cores                      # dst nodes per core
        self.T = math.ceil(self.npc / P)             # real dst tiles per core
        self.T_pad = _round_up(self.T, G)
        self.n_groups = self.T_pad // G
        nseg = self.T_pad * 4

        src = src.astype(np.int64)
        dst = dst.astype(np.int64)

        per_core = []
        max_cnt = 0
        for c in range(n_cores):
            lo = c * self.npc
            sel = (dst >= lo) & (dst < lo + self.npc)
            s_c = src[sel]
            dl = dst[sel] - lo                        # local dst id
            t_all = dl >> 7                           # dst tile
            q_all = s_c // self.Qsz                   # src quarter
            key = t_all * 4 + q_all
            order = np.argsort(key, kind="stable")
            s_c, dl, key = s_c[order], dl[order], key[order]
            cnt = np.bincount(key, minlength=nseg)
            max_cnt = max(max_cnt, int(cnt.max()))
            per_core.append((s_c, dl, key, cnt))

        self.S = max(1, math.ceil(max_cnt / P))      # chunks per segment
        S, G_, Qsz = self.S, G, self.Qsz
        self.ncols = 4 * G * S                       # chunk columns per group
        slots_seg = S * P

        self.gidx = []    # [n_groups*4*128, G*S*8] int16
        self.gdl = []     # [n_groups*128, ncols]   bf16
        for c in range(n_cores):
            s_c, dl, key, cnt = per_core[c]
            flat_idx = np.zeros(nseg * slots_seg, np.int16)
            flat_dl = np.full(nseg * slots_seg, PAD_DST, np.float32)
            starts = np.concatenate([[0], np.cumsum(cnt)])[:-1]
            # position of each edge inside the padded segment layout
            pos_in_seg = np.arange(len(s_c)) - starts[key]
            base = key * slots_seg
            pos = base + pos_in_seg
            q_of_edge = key % 4
            flat_idx[pos] = (s_c - q_of_edge * Qsz).astype(np.int16)
            flat_dl[pos] = (dl & 127).astype(np.float32)

            # flat layout is segment-major: seg = t*4+q, inside: s*128+p.
            # regroup to gather order: per (g, q): (t_loc, s, p)
            fi = flat_idx.reshape(self.T_pad, 4, S, P)
            fd = flat_dl.reshape(self.T_pad, 4, S, P)
            # -> [n_groups, G, 4, S, P] -> [n_groups, 4, G, S, P]
            fi = fi.reshape(self.n_groups, G_, 4, S, P).transpose(0, 2, 1, 3, 4)
            fd = fd.reshape(self.n_groups, G_, 4, S, P).transpose(0, 2, 1, 3, 4)

            # gather idx arrays: flat i = (t_loc*S+s)*128+p ; wrapped [128, i/16]
            fi2 = fi.reshape(self.n_groups, 4, G_ * S * P)
            w = fi2.reshape(self.n_groups, 4, G_ * S * 8, 16)
            w = np.transpose(w, (0, 1, 3, 2))              # [g, 4, 16, cols16]
            w = np.tile(w, (1, 1, 8, 1))                   # replicate to 128
            self.gidx.append(
                np.ascontiguousarray(w.reshape(self.n_groups * 4 * P, G_ * S * 8))
            )

            # dst_local columns: group buffer col c = q*(G*S)+t_loc*S+s
            fcol = fd.reshape(self.n_groups, self.ncols, P)   # [g, c, p]
            gdl = np.transpose(fcol, (0, 2, 1))               # [g, p, c]
            self.gdl.append(
                np.ascontiguousarray(
                    gdl.reshape(self.n_groups * P, self.ncols)
                ).astype(ml_dtypes.bfloat16)
            )


# --------------------------------------------------------------------------
# device kernel builder (shared by both layers)
# --------------------------------------------------------------------------

def build_layer_kernel(es: EdgeStruct, layer: int, with_bias: bool):
    """layer 1: tbl row [xh1(128)|e_src1(8)|junk], 256 bf16 elems,
               heads=8, csz=16, epilogue = softmax-div + ELU + transpose out.
       layer 2: tbl row [xh2(40)|e_src2(1)|junk], 128 bf16 elems, heads=1,
               csz=40, epilogue = softmax-div, row-major f32 out."""
    Npad, T_pad, G, S, ncols = es.Npad, es.T_pad, es.G, es.S, es.ncols
    n_groups, Qsz = es.n_groups, es.Qsz
    if layer == 1:
        ELEM, H, CSZ = 256, 8, 16
    else:
        ELEM, H, CSZ = 128, 1, 40
    # self-loop edges are not in the edge lists; their contribution is added
    # analytically in the tile epilogue from the core's own-node rows.
    MW = H * CSZ                      # message width (128 / 40)
    AW = MW + H                       # [msg | ea] width (136 / 41)
    WCOLS = AW                        # [W | W@a_src_blockdiag]
    WB = WCOLS + H                    # + W@a_dst_blockdiag
    GS = G * S
    NC4 = 4 * GS                      # == ncols

    nc = bacc.Bacc("TRN2", target_bir_lowering=False, debug=False,
                   num_devices=es.n_cores, num_swdge_queues=4)
    ap = {}
    ap["xT"] = nc.dram_tensor("xT", [P, Npad], BF16, kind="ExternalInput").ap()
    ap["xTm"] = nc.dram_tensor("xTm", [P, T_pad * P], BF16,
                               kind="ExternalInput").ap()
    ap["wboth"] = nc.dram_tensor("wboth", [P, WB], BF16,
                                 kind="ExternalInput").ap()
    ap["gidx"] = nc.dram_tensor("gidx", [n_groups * 4 * P, GS * 8], I16,
                                kind="ExternalInput").ap()
    ap["gdl"] = nc.dram_tensor("gdl", [n_groups * P, ncols], BF16,
                               kind="ExternalInput").ap()
    ap["iota_bf"] = nc.dram_tensor("iota_bf", [P, P], BF16,
                                   kind="ExternalInput").ap()
    ap["iota_rep"] = nc.dram_tensor("iota_rep", [P, 4 * S * P], BF16,
                                    kind="ExternalInput").ap()
    ap["idn_bf"] = nc.dram_tensor("idn_bf", [P, P], BF16,
                                  kind="ExternalInput").ap()
    if with_bias:
        ap["ones_bf"] = nc.dram_tensor("ones_bf", [1, P], BF16,
                                       kind="ExternalInput").ap()
        ap["brow"] = nc.dram_tensor("brow", [1, WB], BF16,
                                    kind="ExternalInput").ap()
    if layer == 1:
        out_ap = nc.dram_tensor("hT", [P, T_pad * P], BF16,
                                kind="ExternalOutput").ap()
    else:
        out_ap = nc.dram_tensor("logits", [T_pad * P, MW], F32,
                                kind="ExternalOutput").ap()
    # one table tensor per src quarter so quarter-q gathers only depend on
    # quarter-q prepass writes (overlaps gathers with the prepass tail)
    tblq = [nc.dram_tensor(f"tbl{q}", [Qsz, ELEM], BF16, kind="Internal").ap()
            for q in range(4)]
    own_tbl = nc.dram_tensor("own_tbl", [T_pad * P, WCOLS], BF16,
                             kind="Internal").ap()

    with tile.TileContext(nc) as tc, ExitStack() as ctx:
        cpool = ctx.enter_context(tc.tile_pool(name="consts", bufs=1))

        # ---- constants ----
        wboth = cpool.tile([P, WB], BF16)
        nc.sync.dma_start(wboth[:], ap["wboth"])
        iota_bf = cpool.tile([P, P], BF16)
        nc.sync.dma_start(iota_bf[:], ap["iota_bf"])
        iota_rep = cpool.tile([P, 4 * S * P], BF16)
        nc.sync.dma_start(iota_rep[:], ap["iota_rep"])
        idn_bf = cpool.tile([P, P], BF16)
        nc.sync.dma_start(idn_bf[:], ap["idn_bf"])
        if with_bias:
            ones_bf = cpool.tile([1, P], BF16)
            nc.sync.dma_start(ones_bf[:], ap["ones_bf"])
            brow = cpool.tile([1, WB], BF16)
            nc.sync.dma_start(brow[:], ap["brow"])
        edst_sb = cpool.tile([P, T_pad * H], BF16)

        with tc.tile_pool(name="pre_sb", bufs=4) as psb, \
                tc.tile_pool(name="pre_ps", bufs=4, space="PSUM") as pps:
            # ---- pre-pass A: full feature table [xh | e_src]; octets of
            # node tiles per load/store, pairs per PSUM bank, copies
            # alternating DVE/ACT to amortize per-instruction overheads ----
            NTq = Qsz // P
            for q in range(4):
                for i in range(0, NTq, 8):
                    w8 = min(8, NTq - i)
                    gt = q * NTq + i
                    xt8 = psb.tile([P, 8 * P], BF16, tag="xt")
                    nc.sync.dma_start(xt8[:, 0:w8 * P],
                                      ap["xT"][:, gt * P:(gt + w8) * P])
                    ot8 = psb.tile([P, 8 * WCOLS], BF16, tag="ot")
                    for j2 in range(w8 // 2):
                        ppt = pps.tile([P, 2 * WCOLS], F32, tag="ppt")
                        for jj in range(2):
                            j = 2 * j2 + jj
                            nc.tensor.matmul(
                                out=ppt[:, jj * WCOLS:(jj + 1) * WCOLS],
                                lhsT=xt8[:, j * P:(j + 1) * P],
                                rhs=wboth[:, 0:WCOLS],
                                start=True, stop=not with_bias,
                                skip_group_check=True)
                            if with_bias:
                                # bias row: feature columns get +bias (score
                                # columns of brow are zero); since sum(att)=1
                                # this reproduces "+ bias" after aggregation.
                                nc.tensor.matmul(
                                    out=ppt[:, jj * WCOLS:(jj + 1) * WCOLS],
                                    lhsT=ones_bf[:], rhs=brow[:, 0:WCOLS],
                                    start=False, stop=True,
                                    skip_group_check=True)
                        dst8 = ot8[:, j2 * 2 * WCOLS:(j2 + 1) * 2 * WCOLS]
                        if j2 % 2 == 0:
                            nc.vector.tensor_copy(out=dst8, in_=ppt[:])
                        else:
                            nc.scalar.activation(out=dst8, in_=ppt[:],
                                                 func=AF.Copy)
                    dst = tblq[q][i * P:(i + w8) * P, 0:WCOLS] \
                        .rearrange("(j p) w -> p j w", p=P)
                    nc.sync.dma_start(dst, ot8[:, 0:w8 * WCOLS].rearrange(
                        "p (j w) -> p j w", j=w8))

            # ---- pre-pass B: own rows [xh|e_src] (DRAM) and e_dst (SBUF),
            # pairs of tiles per load/psum/copy/store ----
            for t in range(0, T_pad, 2):
                xt2 = psb.tile([P, 2 * P], BF16, tag="xt2")
                nc.sync.dma_start(xt2[:], ap["xTm"][:, t * P:(t + 2) * P])
                po = pps.tile([P, 2 * WB], F32, tag="po")
                po3 = po[:].rearrange("p (j w) -> p j w", w=WB)
                for j in range(2):
                    nc.tensor.matmul(out=po[:, j * WB:(j + 1) * WB],
                                     lhsT=xt2[:, j * P:(j + 1) * P],
                                     rhs=wboth[:],
                                     start=True, stop=not with_bias,
                                     skip_group_check=True)
                    if with_bias:
                        nc.tensor.matmul(out=po[:, j * WB:(j + 1) * WB],
                                         lhsT=ones_bf[:], rhs=brow[:],
                                         start=False, stop=True,
                                         skip_group_check=True)
                oo = psb.tile([P, 2 * WCOLS], BF16, tag="oo")
                nc.vector.tensor_copy(
                    out=oo[:].rearrange("p (j w) -> p j w", j=2),
                    in_=po3[:, :, 0:WCOLS])
                nc.sync.dma_start(
                    own_tbl[t * P:(t + 2) * P, :]
                    .rearrange("(j p) w -> p j w", p=P),
                    oo[:].rearrange("p (j w) -> p j w", j=2))
                nc.scalar.activation(
                    out=edst_sb[:, t * H:(t + 2) * H]
                    .rearrange("p (j h) -> p j h", j=2),
                    in_=po3[:, :, WCOLS:WB], func=AF.Copy)

        # ---- edge pass ----
        sb = ctx.enter_context(tc.tile_pool(name="sb", bufs=2))
        gbp = ctx.enter_context(tc.tile_pool(name="gbuf", bufs=2))
        pacc = ctx.enter_context(tc.tile_pool(name="pacc", bufs=2,
                                              space="PSUM"))
        palp = ctx.enter_context(tc.tile_pool(name="palp", bufs=2,
                                              space="PSUM"))
        psdt = ctx.enter_context(tc.tile_pool(name="psdt", bufs=2,
                                              space="PSUM"))
        if layer == 1:
            ptp = ctx.enter_context(tc.tile_pool(name="ptp", bufs=2,
                                                 space="PSUM"))
        NSEG = 4 * S                  # chunks per tile
        for g in range(n_groups):
            # NOTE: the g=0 barrier is load-bearing — removing it (to
            # overlap quarter-0 gathers with the prepass tail) wedges the
            # device (NRT_EXEC_UNIT_UNRECOVERABLE).
            if g % BARRIER_EVERY == 0:
                tc.strict_bb_all_engine_barrier()
            gb = gbp.tile([P, ncols * ELEM], BF16, tag="gb")
            gb3 = gb[:].rearrange("p (c k) -> p c k", k=ELEM)
            idxs = sb.tile([P, 4 * GS * 8], I16, tag="idx")
            nc.sync.dma_start(
                idxs[:].rearrange("p (q j) -> p q j", q=4),
                ap["gidx"][g * 4 * P:(g + 1) * 4 * P, :]
                .rearrange("(q p) j -> p q j", p=P))
            dlt = sb.tile([P, ncols], BF16, tag="dl")
            nc.sync.dma_start(dlt[:], ap["gdl"][g * P:(g + 1) * P, :])
            own_g = sb.tile([P, G * WCOLS], BF16, tag="own")
            nc.sync.dma_start(
                own_g[:].rearrange("p (j w) -> p j w", j=G),
                own_tbl[g * G * P:(g + 1) * G * P, :]
                .rearrange("(j p) w -> p j w", p=P))
            if layer == 1:
                hT_g = sb.tile([P, G * P], BF16, tag="hTg")
            else:
                lg_g = sb.tile([P, G * MW], F32, tag="lgg")
            gq = 0
            for q in range(4):
                for c0 in range(0, GS, MAXGC):
                    c1 = min(c0 + MAXGC, GS)
                    nc.gpsimd.dma_gather(
                        out_ap=gb3[:, q * GS + c0:q * GS + c1, :],
                        in_ap=tblq[q],
                        idxs_ap=idxs[:, (q * GS + c0) * 8:(q * GS + c1) * 8],
                        num_idxs=(c1 - c0) * P,
                        num_idxs_reg=(c1 - c0) * P,
                        elem_size=ELEM,
                        queue_num=gq % 4,
                    )
                    gq += 1
            dlt4 = dlt[:].rearrange("p (q c) -> p q c", q=4)
            for t_loc in range(G):
                t = g * G + t_loc
                # ---- selection matrices: one is_equal builds all 4S seT
                # blocks of this tile; PE transposes give the sdT blocks ----
                seT = sb.tile([P, NSEG * P], BF16, tag="seT")
                seT4 = seT[:].rearrange("p (q s d) -> p q s d", q=4, d=P)
                nc.vector.tensor_tensor(
                    out=seT4,
                    in0=dlt4[:, :, t_loc * S:(t_loc + 1) * S]
                    .rearrange("p q (s o) -> p q s o", o=1)
                    .to_broadcast([P, 4, S, P]),
                    in1=iota_bf[:].rearrange("p (a b d) -> p a b d", a=1, b=1)
                    .to_broadcast([P, 4, S, P]),
                    op=ALU.is_equal)
                sdt = sb.tile([P, NSEG * P], BF16, tag="sdt")
                for b0 in range(0, NSEG, 4):
                    b1 = min(b0 + 4, NSEG)
                    ps = psdt.tile([P, 512], F32, tag="ps")
                    for j in range(b0, b1):
                        nc.tensor.matmul(
                            out=ps[:, (j - b0) * P:(j - b0 + 1) * P],
                            lhsT=seT[:, j * P:(j + 1) * P], rhs=idn_bf[:],
                            start=True, stop=True, skip_group_check=True)
                    if (b0 // 4) % 2 == 0:
                        nc.scalar.activation(
                            out=sdt[:, b0 * P:b1 * P],
                            in_=ps[:, 0:(b1 - b0) * P], func=AF.Copy)
                    else:
                        nc.vector.tensor_copy(
                            out=sdt[:, b0 * P:b1 * P],
                            in_=ps[:, 0:(b1 - b0) * P])
                # ---- per-edge e_dst then alpha = lrelu(e_src + e_dst) ----
                pa = palp.tile([P, NSEG * H], F32, tag="pa")
                for j in range(NSEG):
                    nc.tensor.matmul(
                        out=pa[:, j * H:(j + 1) * H],
                        lhsT=sdt[:, j * P:(j + 1) * P],
                        rhs=edst_sb[:, t * H:(t + 1) * H],
                        start=True, stop=True, skip_group_check=True)
                albuf = sb.tile([P, NSEG * H], F32, tag="al")
                nc.vector.tensor_tensor(
                    out=albuf[:].rearrange("p (q s h) -> p q s h", q=4, h=H),
                    in0=pa[:].rearrange("p (q s h) -> p q s h", q=4, h=H),
                    in1=gb3[:, :, MW:MW + H]
                    .rearrange("p (q c) h -> p q c h", q=4)
                    [:, :, t_loc * S:(t_loc + 1) * S, :],
                    op=ALU.add)
                if USE_LRELU:
                    nc.scalar.activation(out=albuf[:], in_=albuf[:],
                                         func=AF.Lrelu, alpha=NEG_SLOPE)
                else:
                    al2 = sb.tile([P, NSEG * H], F32, tag="al2")
                    nc.vector.tensor_scalar_mul(out=al2[:], in0=albuf[:],
                                                scalar1=NEG_SLOPE)
                    nc.vector.tensor_tensor(out=albuf[:], in0=albuf[:],
                                            in1=al2[:], op=ALU.max)
                mea = sb.tile([P, NSEG * AW], BF16, tag="mea")
                mea3 = mea[:].rearrange("p (c w) -> p c w", w=AW)
                nc.scalar.activation(out=mea3[:, :, MW:MW + H],
                                     in_=albuf[:].rearrange(
                                         "p (c h) -> p c h", h=H),
                                     func=AF.Exp)
                # ---- messages msg = xh * ea (broadcast over channels) ----
                for q in range(4):
                    ea_q = mea3[:, q * S:(q + 1) * S, MW:MW + H] \
                        .rearrange("p s (h o) -> p s h o", o=1) \
                        .to_broadcast([P, S, H, CSZ])
                    xh_q = gb3[:, q * GS + t_loc * S:q * GS + (t_loc + 1) * S,
                               0:MW].rearrange("p s (h c) -> p s h c", c=CSZ)
                    msg_q = mea3[:, q * S:(q + 1) * S, 0:MW] \
                        .rearrange("p s (h c) -> p s h c", c=CSZ)
                    nc.vector.tensor_tensor(out=msg_q, in0=ea_q, in1=xh_q,
                                            op=ALU.mult)
                # ---- segment-reduce into the tile accumulator ----
                acc = pacc.tile([P, AW], F32, tag="acc")
                for j in range(NSEG):
                    nc.tensor.matmul(out=acc[:],
                                     lhsT=seT[:, j * P:(j + 1) * P],
                                     rhs=mea[:, j * AW:(j + 1) * AW],
                                     start=(j == 0), stop=(j == NSEG - 1),
                                     skip_group_check=True)
                # ---- tile epilogue (adds analytic self-loop term) ----
                own = own_g[:, t_loc * WCOLS:(t_loc + 1) * WCOLS]
                als = sb.tile([P, H], F32, tag="als")
                nc.vector.tensor_tensor(out=als[:], in0=own[:, MW:WCOLS],
                                        in1=edst_sb[:, t * H:(t + 1) * H],
                                        op=ALU.add)
                eas = sb.tile([P, H], F32, tag="eas")
                if USE_LRELU:
                    nc.scalar.activation(out=eas[:], in_=als[:], func=AF.Lrelu,
                                         alpha=NEG_SLOPE)
                else:
                    als2 = sb.tile([P, H], F32, tag="als2")
                    nc.vector.tensor_scalar_mul(out=als2[:], in0=als[:],
                                                scalar1=NEG_SLOPE)
                    nc.vector.tensor_tensor(out=eas[:], in0=als[:],
                                            in1=als2[:], op=ALU.max)
                nc.scalar.activation(out=eas[:], in_=eas[:], func=AF.Exp)
                # self message: own xh columns include +bias, matching tbl.
                smsg = sb.tile([P, MW], F32, tag="smsg")
                if H == 1:
                    nc.vector.tensor_tensor(
                        out=smsg[:], in0=eas[:, 0:1].to_broadcast([P, MW]),
                        in1=own[:, 0:MW], op=ALU.mult)
                else:
                    nc.vector.tensor_tensor(
                        out=smsg[:].rearrange("p (h c) -> p h c", c=CSZ),
                        in0=eas[:].rearrange("p (h o) -> p h o", o=1)
                        .to_broadcast([P, H, CSZ]),
                        in1=own[:, 0:MW].rearrange("p (h c) -> p h c", c=CSZ),
                        op=ALU.mult)
                unorm = sb.tile([P, MW], F32, tag="unorm")
                nc.vector.tensor_tensor(out=unorm[:], in0=acc[:, 0:MW],
                                        in1=smsg[:], op=ALU.add)
                den = sb.tile([P, H], F32, tag="den")
                # (+EPS dropped: den >= exp(lrelu(.)) > 0 always)
                nc.vector.tensor_tensor(out=den[:], in0=acc[:, MW:AW],
                                        in1=eas[:], op=ALU.add)
                rec = sb.tile([P, H], F32, tag="rec")
                nc.vector.reciprocal(out=rec[:], in_=den[:])
                if layer == 1:
                    otile = sb.tile([P, MW], F32, tag="otile")
                else:
                    otile = lg_g[:, t_loc * MW:(t_loc + 1) * MW]
                if H == 1:
                    nc.vector.tensor_tensor(
                        out=otile, in0=rec[:, 0:1].to_broadcast([P, MW]),
                        in1=unorm[:], op=ALU.mult)
                else:
                    rec3 = rec[:].rearrange("p (h o) -> p h o", o=1) \
                        .to_broadcast([P, H, CSZ])
                    acc3 = unorm[:].rearrange("p (h c) -> p h c", c=CSZ)
                    ot3 = otile[:].rearrange("p (h c) -> p h c", c=CSZ)
                    nc.vector.tensor_tensor(out=ot3, in0=rec3, in1=acc3,
                                            op=ALU.mult)
                if layer == 1:
                    # ELU: relu(x) + exp(min(x,0)) - 1, then transpose out
                    tmp = sb.tile([P, MW], F32, tag="tmp")
                    nc.vector.tensor_scalar_min(out=tmp[:], in0=otile[:],
                                                scalar1=0.0)
                    nc.scalar.activation(out=tmp[:], in_=tmp[:], func=AF.Exp)
                    nc.scalar.activation(out=otile[:], in_=otile[:],
                                         func=AF.Relu)
                    nc.vector.tensor_tensor(out=otile[:], in0=tmp[:],
                                            in1=otile[:], op=ALU.add)
                    obf = sb.tile([P, MW], BF16, tag="obf")
                    nc.scalar.activation(out=obf[:], in_=otile[:],
                                         func=AF.Copy, bias=-1.0)
                    tp = ptp.tile([P, P], F32, tag="tp")
                    nc.tensor.matmul(out=tp[:], lhsT=obf[:], rhs=idn_bf[:],
                                     start=True, stop=True,
                                     skip_group_check=True)
                    nc.scalar.activation(
                        out=hT_g[:, t_loc * P:(t_loc + 1) * P],
                        in_=tp[:], func=AF.Copy)
            if layer == 1:
                nc.sync.dma_start(out_ap[:, g * G * P:(g + 1) * G * P],
                                  hT_g[:])
            else:
                nc.sync.dma_start(
                    out_ap[g * G * P:(g + 1) * G * P, :]
                    .rearrange("(j p) w -> p j w", p=P),
                    lg_g[:].rearrange("p (j w) -> p j w", j=G))

    nc.compile()
    return nc


# --------------------------------------------------------------------------
# host orchestration
# --------------------------------------------------------------------------

def _consts_inputs(es):
    iota = np.arange(P, dtype=np.float32)
    iota_bf = np.tile(iota.astype(ml_dtypes.bfloat16)[None, :], (P, 1))
    return {
        "iota_bf": iota_bf,
        "iota_rep": np.ascontiguousarray(np.tile(iota_bf, (1, 4 * es.S))),
        "idn_bf": np.eye(P, dtype=ml_dtypes.bfloat16),
    }


def _blockdiag(att):
    """[H, C] attention vector -> [H*C, H] block-diagonal matrix."""
    H, C = att.shape
    out = np.zeros((H * C, H), np.float32)
    for h in range(H):
        out[h * C:(h + 1) * C, h] = att[h]
    return out


def _layer_inputs(es, xT_full, W, a_src, a_dst, b, n_cores, consts):
    """Build per-core input maps for one layer. xT_full: [P, Npad] bf16."""
    W = np.asarray(W, np.float32)
    wboth = np.concatenate(
        [W, W @ _blockdiag(np.asarray(a_src, np.float32)),
         W @ _blockdiag(np.asarray(a_dst, np.float32))],
        axis=1).astype(ml_dtypes.bfloat16)
    b = np.asarray(b, np.float32)
    with_bias = bool(np.any(b))
    T_pad, npc = es.T_pad, es.npc
    in_maps = []
    for c in range(n_cores):
        xTm = np.zeros((P, T_pad * P), ml_dtypes.bfloat16)
        xTm[:, :npc] = xT_full[:, c * npc:(c + 1) * npc]
        m = {
            "xT": xT_full, "xTm": xTm, "wboth": wboth,
            "gidx": es.gidx[c], "gdl": es.gdl[c],
            **consts,
        }
        if with_bias:
            brow = np.zeros((1, wboth.shape[1]), np.float32)
            brow[0, :len(b)] = b
            m["brow"] = brow.astype(ml_dtypes.bfloat16)
            m["ones_bf"] = np.ones((1, P), ml_dtypes.bfloat16)
        in_maps.append(m)
    return in_maps, with_bias


def run_gat(x, edge_index, W1, att_src1, att_dst1, b1, W2, att_src2, att_dst2,
            b2, N, n_cores, G=5, es=None, verbose=False):
    x = np.asarray(x, np.float32)
    src = np.asarray(edge_index[0]).astype(np.int64)
    dst = np.asarray(edge_index[1]).astype(np.int64)
    # self-loops are handled analytically inside the kernel epilogue

    if es is None:
        es = EdgeStruct(src, dst, N, n_cores, G=G)
    npc, Npad, T_pad = es.npc, es.Npad, es.T_pad

    consts = _consts_inputs(es)
    xT = np.zeros((P, Npad), ml_dtypes.bfloat16)
    xT[:, :N] = x.T.astype(ml_dtypes.bfloat16)

    in_maps, wb1 = _layer_inputs(es, xT, W1, att_src1, att_dst1, b1,
                                 n_cores, consts)
    nc1 = build_layer_kernel(es, 1, wb1)
    res1 = run_bass_kernel_spmd(nc1, in_maps, core_ids=list(range(n_cores)))
    hT = np.zeros((P, Npad), ml_dtypes.bfloat16)
    for c in range(n_cores):
        hT[:, c * npc:(c + 1) * npc] = res1.results[c]["hT"][:, :npc]

    in_maps2, wb2 = _layer_inputs(es, hT, W2, att_src2, att_dst2,
                                  np.zeros(1), n_cores, consts)
    nc2 = build_layer_kernel(es, 2, wb2)
    res2 = run_bass_kernel_spmd(nc2, in_maps2, core_ids=list(range(n_cores)))
    ncls = np.asarray(W2).shape[1]
    out = np.zeros((N, ncls), np.float32)
    for c in range(n_cores):
        out[c * npc:(c + 1) * npc] = res2.results[c]["logits"][:npc, :]
    out += np.asarray(b2, np.float32)[None, :]
    return out


def kernel(x, edge_index, W1, att_src1, att_dst1, b1, W2, att_src2, att_dst2,
           b2):
    N = int(np.asarray(x).shape[0])
    return run_gat(x, edge_index, W1, att_src1, att_dst1, b1, W2, att_src2,
                   att_dst2, b2, N=N, n_cores=8)


# revision 40
# speedup vs baseline: 5967.0877x; 5967.0877x over previous
"""GAT (2-layer, PyG-style) distributed Bass kernel for 8 Trainium2 NeuronCores.

Strategy (graph/data parallel, per sharding hint):
  - Nodes are partitioned into 8 contiguous blocks; core c owns destination
    nodes [c*N/8, (c+1)*N/8) and all edges incident to them (plus self loops).
  - Each layer: every core builds the full node feature table
    tbl[n] = [xh(n) | e_src(n)] in bf16 (redundant compute beats cross-core
    collectives), then processes its destination tiles: one hardware
    dma_gather per (tile-group x src-quarter) fetches per-edge source rows,
    attention coefficients are formed with Lrelu+Exp on the scalar engine
    (no max-subtraction needed: scores are O(1) so exp never overflows;
    softmax is exactly equivalent), and a 0/1 selection-matrix matmul in
    bf16 on the tensor engine performs the per-destination segment
    reduction of [msg | ea] in PSUM.
  - Host reassembles the transposed hidden table h_T from the 8 shards
    (pure data movement), then launch 2 repeats the same structure with
    the 40-wide single-head output layer.

All tensor-engine work is bf16 (1 cycle/row vs 4 for fp32); accumulation
stays fp32 in PSUM. Vector-engine work is batched per destination tile
(one is_equal builds all selection matrices of a tile; alpha/exp/msg-mult
are strided batch ops) to amortize per-instruction overheads.

SPMD constraints force fully uniform static structure across cores: every
(dst-tile x src-quarter) edge segment is padded to S chunks of 128 edges
(pad edges gather row 0 and use an out-of-range dst slot so selection
matrices zero them out). Source indices are split into 4 quarters because
dma_gather indices are int16.
"""

import math
import os
import sys

for _p in ("/opt/trn_rl_repo", "/root/.axon_site/_ro/trn_rl_repo"):
    if os.path.isdir(_p) and _p not in sys.path:
        sys.path.insert(0, _p)

import numpy as np
import ml_dtypes
from contextlib import ExitStack

import concourse.bacc as bacc
import concourse.bass as bass
import concourse.tile as tile
from concourse import mybir
from concourse.bass_utils import run_bass_kernel_spmd

F32 = mybir.dt.float32
BF16 = mybir.dt.bfloat16
I16 = mybir.dt.int16
AF = mybir.ActivationFunctionType
ALU = mybir.AluOpType

NEG_SLOPE = 0.2
EPS = 1e-16
P = 128
PAD_DST = 200.0  # sentinel dst_local for pad edges; never matches iota 0..127
BARRIER_EVERY = 10
USE_LRELU = False   # ACT Lrelu mis-lowers on HW (rel err 6e-2); keep DVE path
MAXGC = 7           # max gather size in 128-idx chunk columns per instruction
                    # (1024 idxs/gather verified on HW; 2048 wedges the core)


# --------------------------------------------------------------------------
# host-side graph preprocessing
# --------------------------------------------------------------------------

def _round_up(a, b):
    return (a + b - 1) // b * b


class EdgeStruct:
    """Uniform SPMD edge layout shared by both layers."""

    def __init__(self, src, dst, N, n_cores, G=5):
        self.N = N
        self.n_cores = n_cores
        self.G = G
        self.Npad = _round_up(N, 512)
        self.Qsz = self.Npad // 4
        assert self.Qsz <= 32767
        assert N % n_cores == 0
        self.npc = N // n_cores                      # dst nodes per core
        self.T = math.ceil(self.npc / P)             # real dst tiles per core
        self.T_pad = _round_up(self.T, G)
        self.n_groups = self.T_pad // G
        nseg = self.T_pad * 4

        src = src.astype(np.int64)
        dst = dst.astype(np.int64)

        per_core = []
        max_cnt = 0
        for c in range(n_cores):
            lo = c * self.npc
            sel = (dst >= lo) & (dst < lo + self.npc)
            s_c = src[sel]
            dl = dst[sel] - lo                        # local dst id
            t_all = dl >> 7                           # dst tile
            q_all = s_c // self.Qsz                   # src quarter
            key = t_all * 4 + q_all
            order = np.argsort(key, kind="stable")
            s_c, dl, key = s_c[order], dl[order], key[order]
            cnt = np.bincount(key, minlength=nseg)
            max_cnt = max(max_cnt, int(cnt.max()))
            per_core.append((s_c, dl, key, cnt))

        self.S = max(1, math.ceil(max_cnt / P))      # chunks per segment
        S, G_, Qsz = self.S, G, self.Qsz
        self.ncols = 4 * G * S                       # chunk columns per group
        slots_seg = S * P

        self.gidx = []    # [n_groups*4*128, G*S*8] int16
        self.gdl = []     # [n_groups*128, ncols]   bf16
        for c in range(n_cores):
            s_c, dl, key, cnt = per_core[c]
            flat_idx = np.zeros(nseg * slots_seg, np.int16)
            flat_dl = np.full(nseg * slots_seg, PAD_DST, np.float32)
            starts = np.concatenate([[0], np.cumsum(cnt)])[:-1]
            # position of each edge inside the padded segment layout
            pos_in_seg = np.arange(len(s_c)) - starts[key]
            base = key * slots_seg
            pos = base + pos_in_seg
            q_of_edge = key % 4
            flat_idx[pos] = (s_c - q_of_edge * Qsz).astype(np.int16)
            flat_dl[pos] = (dl & 127).astype(np.float32)

            # flat layout is segment-major: seg = t*4+q, inside: s*128+p.
            # regroup to gather order: per (g, q): (t_loc, s, p)
            fi = flat_idx.reshape(self.T_pad, 4, S, P)
            fd = flat_dl.reshape(self.T_pad, 4, S, P)
            # -> [n_groups, G, 4, S, P] -> [n_groups, 4, G, S, P]
            fi = fi.reshape(self.n_groups, G_, 4, S, P).transpose(0, 2, 1, 3, 4)
            fd = fd.reshape(self.n_groups, G_, 4, S, P).transpose(0, 2, 1, 3, 4)

            # gather idx arrays: flat i = (t_loc*S+s)*128+p ; wrapped [128, i/16]
            fi2 = fi.reshape(self.n_groups, 4, G_ * S * P)
            w = fi2.reshape(self.n_groups, 4, G_ * S * 8, 16)
            w = np.transpose(w, (0, 1, 3, 2))              # [g, 4, 16, cols16]
            w = np.tile(w, (1, 1, 8, 1))                   # replicate to 128
            self.gidx.append(
                np.ascontiguousarray(w.reshape(self.n_groups * 4 * P, G_ * S * 8))
            )

            # dst_local columns: group buffer col c = q*(G*S)+t_loc*S+s
            fcol = fd.reshape(self.n_groups, self.ncols, P)   # [g, c, p]
            gdl = np.transpose(fcol, (0, 2, 1))               # [g, p, c]
            self.gdl.append(
                np.ascontiguousarray(
                    gdl.reshape(self.n_groups * P, self.ncols)
                ).astype(ml_dtypes.bfloat16)
            )


# --------------------------------------------------------------------------
# device kernel builder (shared by both layers)
# --------------------------------------------------------------------------

def build_layer_kernel(es: EdgeStruct, layer: int, with_bias: bool):
    """layer 1: tbl row [xh1(128)|e_src1(8)|junk], 256 bf16 elems,
               heads=8, csz=16, epilogue = softmax-div + ELU + transpose out.
       layer 2: tbl row [xh2(40)|e_src2(1)|junk], 128 bf16 elems, heads=1,
               csz=40, epilogue = softmax-div, row-major f32 out."""
    Npad, T_pad, G, S, ncols = es.Npad, es.T_pad, es.G, es.S, es.ncols
    n_groups, Qsz = es.n_groups, es.Qsz
    if layer == 1:
        ELEM, H, CSZ = 256, 8, 16
    else:
        ELEM, H, CSZ = 128, 1, 40
    # self-loop edges are not in the edge lists; their contribution is added
    # analytically in the tile epilogue from the core's own-node rows.
    MW = H * CSZ                      # message width (128 / 40)
    AW = MW + H                       # [msg | ea] width (136 / 41)
    WCOLS = AW                        # [W | W@a_src_blockdiag]
    WB = WCOLS + H                    # + W@a_dst_blockdiag
    GS = G * S
    NC4 = 4 * GS                      # == ncols

    nc = bacc.Bacc("TRN2", target_bir_lowering=False, debug=False,
                   num_devices=es.n_cores, num_swdge_queues=4)
    ap = {}
    ap["xT"] = nc.dram_tensor("xT", [P, Npad], BF16, kind="ExternalInput").ap()
    ap["xTm"] = nc.dram_tensor("xTm", [P, T_pad * P], BF16,
                               kind="ExternalInput").ap()
    ap["wboth"] = nc.dram_tensor("wboth", [P, WB], BF16,
                                 kind="ExternalInput").ap()
    ap["gidx"] = nc.dram_tensor("gidx", [n_groups * 4 * P, GS * 8], I16,
                                kind="ExternalInput").ap()
    ap["gdl"] = nc.dram_tensor("gdl", [n_groups * P, ncols], BF16,
                               kind="ExternalInput").ap()
    ap["iota_bf"] = nc.dram_tensor("iota_bf", [P, P], BF16,
                                   kind="ExternalInput").ap()
    ap["iota_rep"] = nc.dram_tensor("iota_rep", [P, 4 * S * P], BF16,
                                    kind="ExternalInput").ap()
    ap["idn_bf"] = nc.dram_tensor("idn_bf", [P, P], BF16,
                                  kind="ExternalInput").ap()
    if with_bias:
        ap["ones_bf"] = nc.dram_tensor("ones_bf", [1, P], BF16,
                                       kind="ExternalInput").ap()
        ap["brow"] = nc.dram_tensor("brow", [1, WB], BF16,
                                    kind="ExternalInput").ap()
    if layer == 1:
        out_ap = nc.dram_tensor("hT", [P, T_pad * P], BF16,
                                kind="ExternalOutput").ap()
    else:
        out_ap = nc.dram_tensor("logits", [T_pad * P, MW], F32,
                                kind="ExternalOutput").ap()
    # one table tensor per src quarter so quarter-q gathers only depend on
    # quarter-q prepass writes (overlaps gathers with the prepass tail)
    tblq = [nc.dram_tensor(f"tbl{q}", [Qsz, ELEM], BF16, kind="Internal").ap()
            for q in range(4)]
    own_tbl = nc.dram_tensor("own_tbl", [T_pad * P, WCOLS], BF16,
                             kind="Internal").ap()

    with tile.TileContext(nc) as tc, ExitStack() as ctx:
        cpool = ctx.enter_context(tc.tile_pool(name="consts", bufs=1))

        # ---- constants ----
        wboth = cpool.tile([P, WB], BF16)
        nc.sync.dma_start(wboth[:], ap["wboth"])
        iota_bf = cpool.tile([P, P], BF16)
        nc.sync.dma_start(iota_bf[:], ap["iota_bf"])
        iota_rep = cpool.tile([P, 4 * S * P], BF16)
        nc.sync.dma_start(iota_rep[:], ap["iota_rep"])
        idn_bf = cpool.tile([P, P], BF16)
        nc.sync.dma_start(idn_bf[:], ap["idn_bf"])
        if with_bias:
            ones_bf = cpool.tile([1, P], BF16)
            nc.sync.dma_start(ones_bf[:], ap["ones_bf"])
            brow = cpool.tile([1, WB], BF16)
            nc.sync.dma_start(brow[:], ap["brow"])
        edst_sb = cpool.tile([P, T_pad * H], BF16)

        with tc.tile_pool(name="pre_sb", bufs=4) as psb, \
                tc.tile_pool(name="pre_ps", bufs=4, space="PSUM") as pps:
            # ---- pre-pass A: full feature table [xh | e_src]; octets of
            # node tiles per load/store, pairs per PSUM bank, copies
            # alternating DVE/ACT to amortize per-instruction overheads ----
            NTq = Qsz // P
            for q in range(4):
                for i in range(0, NTq, 8):
                    w8 = min(8, NTq - i)
                    gt = q * NTq + i
                    xt8 = psb.tile([P, 8 * P], BF16, tag="xt")
                    nc.sync.dma_start(xt8[:, 0:w8 * P],
                                      ap["xT"][:, gt * P:(gt + w8) * P])
                    ot8 = psb.tile([P, 8 * WCOLS], BF16, tag="ot")
                    for j2 in range(w8 // 2):
                        ppt = pps.tile([P, 2 * WCOLS], F32, tag="ppt")
                        for jj in range(2):
                            j = 2 * j2 + jj
                            nc.tensor.matmul(
                                out=ppt[:, jj * WCOLS:(jj + 1) * WCOLS],
                                lhsT=xt8[:, j * P:(j + 1) * P],
                                rhs=wboth[:, 0:WCOLS],
                                start=True, stop=not with_bias,
                                skip_group_check=True)
                            if with_bias:
                                # bias row: feature columns get +bias (score
                                # columns of brow are zero); since sum(att)=1
                                # this reproduces "+ bias" after aggregation.
                                nc.tensor.matmul(
                                    out=ppt[:, jj * WCOLS:(jj + 1) * WCOLS],
                                    lhsT=ones_bf[:], rhs=brow[:, 0:WCOLS],
                                    start=False, stop=True,
                                    skip_group_check=True)
                        dst8 = ot8[:, j2 * 2 * WCOLS:(j2 + 1) * 2 * WCOLS]
                        # ACT copies keep DVE free during the edge pass
                        nc.scalar.activation(out=dst8, in_=ppt[:],
                                             func=AF.Copy)
                    dst = tblq[q][i * P:(i + w8) * P, 0:WCOLS] \
                        .rearrange("(j p) w -> p j w", p=P)
                    nc.sync.dma_start(dst, ot8[:, 0:w8 * WCOLS].rearrange(
                        "p (j w) -> p j w", j=w8))

            # ---- pre-pass B: own rows [xh|e_src] (DRAM) and e_dst (SBUF),
            # pairs of tiles per load/psum/copy/store ----
            for t in range(0, T_pad, 2):
                xt2 = psb.tile([P, 2 * P], BF16, tag="xt2")
                nc.sync.dma_start(xt2[:], ap["xTm"][:, t * P:(t + 2) * P])
                po = pps.tile([P, 2 * WB], F32, tag="po")
                po3 = po[:].rearrange("p (j w) -> p j w", w=WB)
                for j in range(2):
                    nc.tensor.matmul(out=po[:, j * WB:(j + 1) * WB],
                                     lhsT=xt2[:, j * P:(j + 1) * P],
                                     rhs=wboth[:],
                                     start=True, stop=not with_bias,
                                     skip_group_check=True)
                    if with_bias:
                        nc.tensor.matmul(out=po[:, j * WB:(j + 1) * WB],
                                         lhsT=ones_bf[:], rhs=brow[:],
                                         start=False, stop=True,
                                         skip_group_check=True)
                oo = psb.tile([P, 2 * WCOLS], BF16, tag="oo")
                nc.vector.tensor_copy(
                    out=oo[:].rearrange("p (j w) -> p j w", j=2),
                    in_=po3[:, :, 0:WCOLS])
                nc.sync.dma_start(
                    own_tbl[t * P:(t + 2) * P, :]
                    .rearrange("(j p) w -> p j w", p=P),
                    oo[:].rearrange("p (j w) -> p j w", j=2))
                nc.scalar.activation(
                    out=edst_sb[:, t * H:(t + 2) * H]
                    .rearrange("p (j h) -> p j h", j=2),
                    in_=po3[:, :, WCOLS:WB], func=AF.Copy)

        # ---- edge pass ----
        sb = ctx.enter_context(tc.tile_pool(name="sb", bufs=2))
        gbp = ctx.enter_context(tc.tile_pool(name="gbuf", bufs=2))
        pacc = ctx.enter_context(tc.tile_pool(name="pacc", bufs=2,
                                              space="PSUM"))
        palp = ctx.enter_context(tc.tile_pool(name="palp", bufs=2,
                                              space="PSUM"))
        psdt = ctx.enter_context(tc.tile_pool(name="psdt", bufs=2,
                                              space="PSUM"))
        if layer == 1:
            ptp = ctx.enter_context(tc.tile_pool(name="ptp", bufs=2,
                                                 space="PSUM"))
        NSEG = 4 * S                  # chunks per tile
        for g in range(n_groups):
            # NOTE: the g=0 barrier is load-bearing — removing it (to
            # overlap quarter-0 gathers with the prepass tail) wedges the
            # device (NRT_EXEC_UNIT_UNRECOVERABLE).
            if g % BARRIER_EVERY == 0:
                tc.strict_bb_all_engine_barrier()
            gb = gbp.tile([P, ncols * ELEM], BF16, tag="gb")
            gb3 = gb[:].rearrange("p (c k) -> p c k", k=ELEM)
            idxs = sb.tile([P, 4 * GS * 8], I16, tag="idx")
            nc.sync.dma_start(
                idxs[:].rearrange("p (q j) -> p q j", q=4),
                ap["gidx"][g * 4 * P:(g + 1) * 4 * P, :]
                .rearrange("(q p) j -> p q j", p=P))
            dlt = sb.tile([P, ncols], BF16, tag="dl")
            nc.sync.dma_start(dlt[:], ap["gdl"][g * P:(g + 1) * P, :])
            own_g = sb.tile([P, G * WCOLS], BF16, tag="own")
            nc.sync.dma_start(
                own_g[:].rearrange("p (j w) -> p j w", j=G),
                own_tbl[g * G * P:(g + 1) * G * P, :]
                .rearrange("(j p) w -> p j w", p=P))
            if layer == 1:
                hT_g = sb.tile([P, G * P], BF16, tag="hTg")
            else:
                lg_g = sb.tile([P, G * MW], F32, tag="lgg")
            gq = 0
            for q in range(4):
                for c0 in range(0, GS, MAXGC):
                    c1 = min(c0 + MAXGC, GS)
                    nc.gpsimd.dma_gather(
                        out_ap=gb3[:, q * GS + c0:q * GS + c1, :],
                        in_ap=tblq[q],
                        idxs_ap=idxs[:, (q * GS + c0) * 8:(q * GS + c1) * 8],
                        num_idxs=(c1 - c0) * P,
                        num_idxs_reg=(c1 - c0) * P,
                        elem_size=ELEM,
                        queue_num=gq % 4,
                    )
                    gq += 1
            dlt4 = dlt[:].rearrange("p (q c) -> p q c", q=4)
            for t_loc in range(G):
                t = g * G + t_loc
                # ---- selection matrices: one is_equal builds all 4S seT
                # blocks of this tile; PE transposes give the sdT blocks ----
                seT = sb.tile([P, NSEG * P], BF16, tag="seT")
                seT4 = seT[:].rearrange("p (q s d) -> p q s d", q=4, d=P)
                nc.vector.tensor_tensor(
                    out=seT4,
                    in0=dlt4[:, :, t_loc * S:(t_loc + 1) * S]
                    .rearrange("p q (s o) -> p q s o", o=1)
                    .to_broadcast([P, 4, S, P]),
                    in1=iota_bf[:].rearrange("p (a b d) -> p a b d", a=1, b=1)
                    .to_broadcast([P, 4, S, P]),
                    op=ALU.is_equal)
                sdt = sb.tile([P, NSEG * P], BF16, tag="sdt")
                for b0 in range(0, NSEG, 4):
                    b1 = min(b0 + 4, NSEG)
                    ps = psdt.tile([P, 512], F32, tag="ps")
                    for j in range(b0, b1):
                        nc.tensor.matmul(
                            out=ps[:, (j - b0) * P:(j - b0 + 1) * P],
                            lhsT=seT[:, j * P:(j + 1) * P], rhs=idn_bf[:],
                            start=True, stop=True, skip_group_check=True)
                    # all sdT copies on ACT: DVE is the bottleneck engine
                    nc.scalar.activation(
                        out=sdt[:, b0 * P:b1 * P],
                        in_=ps[:, 0:(b1 - b0) * P], func=AF.Copy)
                # ---- per-edge e_dst then alpha = lrelu(e_src + e_dst) ----
                pa = palp.tile([P, NSEG * H], F32, tag="pa")
                for j in range(NSEG):
                    nc.tensor.matmul(
                        out=pa[:, j * H:(j + 1) * H],
                        lhsT=sdt[:, j * P:(j + 1) * P],
                        rhs=edst_sb[:, t * H:(t + 1) * H],
                        start=True, stop=True, skip_group_check=True)
                albuf = sb.tile([P, NSEG * H], F32, tag="al")
                nc.vector.tensor_tensor(
                    out=albuf[:].rearrange("p (q s h) -> p q s h", q=4, h=H),
                    in0=pa[:].rearrange("p (q s h) -> p q s h", q=4, h=H),
                    in1=gb3[:, :, MW:MW + H]
                    .rearrange("p (q c) h -> p q c h", q=4)
                    [:, :, t_loc * S:(t_loc + 1) * S, :],
                    op=ALU.add)
                if USE_LRELU:
                    nc.scalar.activation(out=albuf[:], in_=albuf[:],
                                         func=AF.Lrelu, alpha=NEG_SLOPE)
                else:
                    al2 = sb.tile([P, NSEG * H], F32, tag="al2")
                    nc.vector.tensor_scalar_mul(out=al2[:], in0=albuf[:],
                                                scalar1=NEG_SLOPE)
                    nc.vector.tensor_tensor(out=albuf[:], in0=albuf[:],
                                            in1=al2[:], op=ALU.max)
                mea = sb.tile([P, NSEG * AW], BF16, tag="mea")
                mea3 = mea[:].rearrange("p (c w) -> p c w", w=AW)
                nc.scalar.activation(out=mea3[:, :, MW:MW + H],
                                     in_=albuf[:].rearrange(
                                         "p (c h) -> p c h", h=H),
                                     func=AF.Exp)
                # ---- messages msg = xh * ea (broadcast over channels) ----
                for q in range(4):
                    ea_q = mea3[:, q * S:(q + 1) * S, MW:MW + H] \
                        .rearrange("p s (h o) -> p s h o", o=1) \
                        .to_broadcast([P, S, H, CSZ])
                    xh_q = gb3[:, q * GS + t_loc * S:q * GS + (t_loc + 1) * S,
                               0:MW].rearrange("p s (h c) -> p s h c", c=CSZ)
                    msg_q = mea3[:, q * S:(q + 1) * S, 0:MW] \
                        .rearrange("p s (h c) -> p s h c", c=CSZ)
                    nc.vector.tensor_tensor(out=msg_q, in0=ea_q, in1=xh_q,
                                            op=ALU.mult)
                # ---- segment-reduce into the tile accumulator ----
                acc = pacc.tile([P, AW], F32, tag="acc")
                for j in range(NSEG):
                    nc.tensor.matmul(out=acc[:],
                                     lhsT=seT[:, j * P:(j + 1) * P],
                                     rhs=mea[:, j * AW:(j + 1) * AW],
                                     start=(j == 0), stop=(j == NSEG - 1),
                                     skip_group_check=True)
                # ---- tile epilogue (adds analytic self-loop term) ----
                own = own_g[:, t_loc * WCOLS:(t_loc + 1) * WCOLS]
                als = sb.tile([P, H], F32, tag="als")
                nc.vector.tensor_tensor(out=als[:], in0=own[:, MW:WCOLS],
                                        in1=edst_sb[:, t * H:(t + 1) * H],
                                        op=ALU.add)
                eas = sb.tile([P, H], F32, tag="eas")
                if USE_LRELU:
                    nc.scalar.activation(out=eas[:], in_=als[:], func=AF.Lrelu,
                                         alpha=NEG_SLOPE)
                else:
                    als2 = sb.tile([P, H], F32, tag="als2")
                    nc.vector.tensor_scalar_mul(out=als2[:], in0=als[:],
                                                scalar1=NEG_SLOPE)
                    nc.vector.tensor_tensor(out=eas[:], in0=als[:],
                                            in1=als2[:], op=ALU.max)
                nc.scalar.activation(out=eas[:], in_=eas[:], func=AF.Exp)
                # self message: own xh columns include +bias, matching tbl.
                smsg = sb.tile([P, MW], F32, tag="smsg")
                if H == 1:
                    nc.vector.tensor_tensor(
                        out=smsg[:], in0=eas[:, 0:1].to_broadcast([P, MW]),
                        in1=own[:, 0:MW], op=ALU.mult)
                else:
                    nc.vector.tensor_tensor(
                        out=smsg[:].rearrange("p (h c) -> p h c", c=CSZ),
                        in0=eas[:].rearrange("p (h o) -> p h o", o=1)
                        .to_broadcast([P, H, CSZ]),
                        in1=own[:, 0:MW].rearrange("p (h c) -> p h c", c=CSZ),
                        op=ALU.mult)
                unorm = sb.tile([P, MW], F32, tag="unorm")
                nc.vector.tensor_tensor(out=unorm[:], in0=acc[:, 0:MW],
                                        in1=smsg[:], op=ALU.add)
                den = sb.tile([P, H], F32, tag="den")
                # (+EPS dropped: den >= exp(lrelu(.)) > 0 always)
                nc.vector.tensor_tensor(out=den[:], in0=acc[:, MW:AW],
                                        in1=eas[:], op=ALU.add)
                rec = sb.tile([P, H], F32, tag="rec")
                nc.vector.reciprocal(out=rec[:], in_=den[:])
                if layer == 1:
                    otile = sb.tile([P, MW], F32, tag="otile")
                else:
                    otile = lg_g[:, t_loc * MW:(t_loc + 1) * MW]
                if H == 1:
                    nc.vector.tensor_tensor(
                        out=otile, in0=rec[:, 0:1].to_broadcast([P, MW]),
                        in1=unorm[:], op=ALU.mult)
                else:
                    rec3 = rec[:].rearrange("p (h o) -> p h o", o=1) \
                        .to_broadcast([P, H, CSZ])
                    acc3 = unorm[:].rearrange("p (h c) -> p h c", c=CSZ)
                    ot3 = otile[:].rearrange("p (h c) -> p h c", c=CSZ)
                    nc.vector.tensor_tensor(out=ot3, in0=rec3, in1=acc3,
                                            op=ALU.mult)
                if layer == 1:
                    # ELU: relu(x) + exp(min(x,0)) - 1, then transpose out
                    tmp = sb.tile([P, MW], F32, tag="tmp")
                    nc.vector.tensor_scalar_min(out=tmp[:], in0=otile[:],
                                                scalar1=0.0)
                    nc.scalar.activation(out=tmp[:], in_=tmp[:], func=AF.Exp)
                    nc.scalar.activation(out=otile[:], in_=otile[:],
                                         func=AF.Relu)
                    nc.vector.tensor_tensor(out=otile[:], in0=tmp[:],
                                            in1=otile[:], op=ALU.add)
                    obf = sb.tile([P, MW], BF16, tag="obf")
                    nc.scalar.activation(out=obf[:], in_=otile[:],
                                         func=AF.Copy, bias=-1.0)
                    tp = ptp.tile([P, P], F32, tag="tp")
                    nc.tensor.matmul(out=tp[:], lhsT=obf[:], rhs=idn_bf[:],
                                     start=True, stop=True,
                                     skip_group_check=True)
                    nc.scalar.activation(
                        out=hT_g[:, t_loc * P:(t_loc + 1) * P],
                        in_=tp[:], func=AF.Copy)
            if layer == 1:
                nc.sync.dma_start(out_ap[:, g * G * P:(g + 1) * G * P],
                                  hT_g[:])
            else:
                nc.sync.dma_start(
                    out_ap[g * G * P:(g + 1) * G * P, :]
                    .rearrange("(j p) w -> p j w", p=P),
                    lg_g[:].rearrange("p (j w) -> p j w", j=G))

    nc.compile()
    return nc


# --------------------------------------------------------------------------
# host orchestration
# --------------------------------------------------------------------------

def _consts_inputs(es):
    iota = np.arange(P, dtype=np.float32)
    iota_bf = np.tile(iota.astype(ml_dtypes.bfloat16)[None, :], (P, 1))
    return {
        "iota_bf": iota_bf,
        "iota_rep": np.ascontiguousarray(np.tile(iota_bf, (1, 4 * es.S))),
        "idn_bf": np.eye(P, dtype=ml_dtypes.bfloat16),
    }


def _blockdiag(att):
    """[H, C] attention vector -> [H*C, H] block-diagonal matrix."""
    H, C = att.shape
    out = np.zeros((H * C, H), np.float32)
    for h in range(H):
        out[h * C:(h + 1) * C, h] = att[h]
    return out


def _layer_inputs(es, xT_full, W, a_src, a_dst, b, n_cores, consts):
    """Build per-core input maps for one layer. xT_full: [P, Npad] bf16."""
    W = np.asarray(W, np.float32)
    wboth = np.concatenate(
        [W, W @ _blockdiag(np.asarray(a_src, np.float32)),
         W @ _blockdiag(np.asarray(a_dst, np.float32))],
        axis=1).astype(ml_dtypes.bfloat16)
    b = np.asarray(b, np.float32)
    with_bias = bool(np.any(b))
    T_pad, npc = es.T_pad, es.npc
    in_maps = []
    for c in range(n_cores):
        xTm = np.zeros((P, T_pad * P), ml_dtypes.bfloat16)
        xTm[:, :npc] = xT_full[:, c * npc:(c + 1) * npc]
        m = {
            "xT": xT_full, "xTm": xTm, "wboth": wboth,
            "gidx": es.gidx[c], "gdl": es.gdl[c],
            **consts,
        }
        if with_bias:
            brow = np.zeros((1, wboth.shape[1]), np.float32)
            brow[0, :len(b)] = b
            m["brow"] = brow.astype(ml_dtypes.bfloat16)
            m["ones_bf"] = np.ones((1, P), ml_dtypes.bfloat16)
        in_maps.append(m)
    return in_maps, with_bias


def run_gat(x, edge_index, W1, att_src1, att_dst1, b1, W2, att_src2, att_dst2,
            b2, N, n_cores, G=5, es=None, verbose=False):
    x = np.asarray(x, np.float32)
    src = np.asarray(edge_index[0]).astype(np.int64)
    dst = np.asarray(edge_index[1]).astype(np.int64)
    # self-loops are handled analytically inside the kernel epilogue

    if es is None:
        es = EdgeStruct(src, dst, N, n_cores, G=G)
    npc, Npad, T_pad = es.npc, es.Npad, es.T_pad

    consts = _consts_inputs(es)
    xT = np.zeros((P, Npad), ml_dtypes.bfloat16)
    xT[:, :N] = x.T.astype(ml_dtypes.bfloat16)

    in_maps, wb1 = _layer_inputs(es, xT, W1, att_src1, att_dst1, b1,
                                 n_cores, consts)
    nc1 = build_layer_kernel(es, 1, wb1)
    res1 = run_bass_kernel_spmd(nc1, in_maps, core_ids=list(range(n_cores)))
    hT = np.zeros((P, Npad), ml_dtypes.bfloat16)
    for c in range(n_cores):
        hT[:, c * npc:(c + 1) * npc] = res1.results[c]["hT"][:, :npc]

    in_maps2, wb2 = _layer_inputs(es, hT, W2, att_src2, att_dst2,
                                  np.zeros(1), n_cores, consts)
    nc2 = build_layer_kernel(es, 2, wb2)
    res2 = run_bass_kernel_spmd(nc2, in_maps2, core_ids=list(range(n_cores)))
    ncls = np.asarray(W2).shape[1]
    out = np.zeros((N, ncls), np.float32)
    for c in range(n_cores):
        out[c * npc:(c + 1) * npc] = res2.results[c]["logits"][:npc, :]
    out += np.asarray(b2, np.float32)[None, :]
    return out


def kernel(x, edge_index, W1, att_src1, att_dst1, b1, W2, att_src2, att_dst2,
           b2):
    N = int(np.asarray(x).shape[0])
    return run_gat(x, edge_index, W1, att_src1, att_dst1, b1, W2, att_src2,
                   att_dst2, b2, N=N, n_cores=8)


# revision 42
# speedup vs baseline: 6242.8566x; 1.0462x over previous
"""GAT (2-layer, PyG-style) distributed Bass kernel for 8 Trainium2 NeuronCores.

Strategy (graph/data parallel, per sharding hint):
  - Nodes are partitioned into 8 contiguous blocks; core c owns destination
    nodes [c*N/8, (c+1)*N/8) and all edges incident to them (plus self loops).
  - Each layer: every core builds the full node feature table
    tbl[n] = [xh(n) | e_src(n)] in bf16 (redundant compute beats cross-core
    collectives), then processes its destination tiles: one hardware
    dma_gather per (tile-group x src-quarter) fetches per-edge source rows,
    attention coefficients are formed with Lrelu+Exp on the scalar engine
    (no max-subtraction needed: scores are O(1) so exp never overflows;
    softmax is exactly equivalent), and a 0/1 selection-matrix matmul in
    bf16 on the tensor engine performs the per-destination segment
    reduction of [msg | ea] in PSUM.
  - Host reassembles the transposed hidden table h_T from the 8 shards
    (pure data movement), then launch 2 repeats the same structure with
    the 40-wide single-head output layer.

All tensor-engine work is bf16 (1 cycle/row vs 4 for fp32); accumulation
stays fp32 in PSUM. Vector-engine work is batched per destination tile
(one is_equal builds all selection matrices of a tile; alpha/exp/msg-mult
are strided batch ops) to amortize per-instruction overheads.

SPMD constraints force fully uniform static structure across cores: every
(dst-tile x src-quarter) edge segment is padded to S chunks of 128 edges
(pad edges gather row 0 and use an out-of-range dst slot so selection
matrices zero them out). Source indices are split into 4 quarters because
dma_gather indices are int16.
"""

import math
import os
import sys

for _p in ("/opt/trn_rl_repo", "/root/.axon_site/_ro/trn_rl_repo"):
    if os.path.isdir(_p) and _p not in sys.path:
        sys.path.insert(0, _p)

import numpy as np
import ml_dtypes
from contextlib import ExitStack

import concourse.bacc as bacc
import concourse.bass as bass
import concourse.tile as tile
from concourse import mybir
from concourse.bass_utils import run_bass_kernel_spmd

F32 = mybir.dt.float32
BF16 = mybir.dt.bfloat16
I16 = mybir.dt.int16
AF = mybir.ActivationFunctionType
ALU = mybir.AluOpType

NEG_SLOPE = 0.2
EPS = 1e-16
P = 128
PAD_DST = 200.0  # sentinel dst_local for pad edges; never matches iota 0..127
BARRIER_EVERY = 10
USE_LRELU = False   # ACT Lrelu mis-lowers on HW (rel err 6e-2); keep DVE path
MAXGC = 7           # max gather size in 128-idx chunk columns per instruction
                    # (1024 idxs/gather verified on HW; 2048 wedges the core)


# --------------------------------------------------------------------------
# host-side graph preprocessing
# --------------------------------------------------------------------------

def _round_up(a, b):
    return (a + b - 1) // b * b


class EdgeStruct:
    """Uniform SPMD edge layout shared by both layers."""

    def __init__(self, src, dst, N, n_cores, G=5):
        self.N = N
        self.n_cores = n_cores
        self.G = G
        self.Npad = _round_up(N, 512)
        self.Qsz = self.Npad // 4
        assert self.Qsz <= 32767
        assert N % n_cores == 0
        self.npc = N // n_cores                      # dst nodes per core
        self.T = math.ceil(self.npc / P)             # real dst tiles per core
        self.T_pad = _round_up(self.T, G)
        self.n_groups = self.T_pad // G
        nseg = self.T_pad * 4

        src = src.astype(np.int64)
        dst = dst.astype(np.int64)

        per_core = []
        max_cnt = 0
        for c in range(n_cores):
            lo = c * self.npc
            sel = (dst >= lo) & (dst < lo + self.npc)
            s_c = src[sel]
            dl = dst[sel] - lo                        # local dst id
            t_all = dl >> 7                           # dst tile
            q_all = s_c // self.Qsz                   # src quarter
            key = t_all * 4 + q_all
            order = np.argsort(key, kind="stable")
            s_c, dl, key = s_c[order], dl[order], key[order]
            cnt = np.bincount(key, minlength=nseg)
            max_cnt = max(max_cnt, int(cnt.max()))
            per_core.append((s_c, dl, key, cnt))

        self.S = max(1, math.ceil(max_cnt / P))      # chunks per segment
        S, G_, Qsz = self.S, G, self.Qsz
        self.ncols = 4 * G * S                       # chunk columns per group
        slots_seg = S * P

        self.gidx = []    # [n_groups*4*128, G*S*8] int16
        self.gdl = []     # [n_groups*128, ncols]   bf16
        for c in range(n_cores):
            s_c, dl, key, cnt = per_core[c]
            flat_idx = np.zeros(nseg * slots_seg, np.int16)
            flat_dl = np.full(nseg * slots_seg, PAD_DST, np.float32)
            starts = np.concatenate([[0], np.cumsum(cnt)])[:-1]
            # position of each edge inside the padded segment layout
            pos_in_seg = np.arange(len(s_c)) - starts[key]
            base = key * slots_seg
            pos = base + pos_in_seg
            q_of_edge = key % 4
            flat_idx[pos] = (s_c - q_of_edge * Qsz).astype(np.int16)
            flat_dl[pos] = (dl & 127).astype(np.float32)

            # flat layout is segment-major: seg = t*4+q, inside: s*128+p.
            # regroup to gather order: per (g, q): (t_loc, s, p)
            fi = flat_idx.reshape(self.T_pad, 4, S, P)
            fd = flat_dl.reshape(self.T_pad, 4, S, P)
            # -> [n_groups, G, 4, S, P] -> [n_groups, 4, G, S, P]
            fi = fi.reshape(self.n_groups, G_, 4, S, P).transpose(0, 2, 1, 3, 4)
            fd = fd.reshape(self.n_groups, G_, 4, S, P).transpose(0, 2, 1, 3, 4)

            # gather idx arrays: flat i = (t_loc*S+s)*128+p ; wrapped [128, i/16]
            fi2 = fi.reshape(self.n_groups, 4, G_ * S * P)
            w = fi2.reshape(self.n_groups, 4, G_ * S * 8, 16)
            w = np.transpose(w, (0, 1, 3, 2))              # [g, 4, 16, cols16]
            w = np.tile(w, (1, 1, 8, 1))                   # replicate to 128
            self.gidx.append(
                np.ascontiguousarray(w.reshape(self.n_groups * 4 * P, G_ * S * 8))
            )

            # dst_local columns: group buffer col c = q*(G*S)+t_loc*S+s
            fcol = fd.reshape(self.n_groups, self.ncols, P)   # [g, c, p]
            gdl = np.transpose(fcol, (0, 2, 1))               # [g, p, c]
            self.gdl.append(
                np.ascontiguousarray(
                    gdl.reshape(self.n_groups * P, self.ncols)
                ).astype(ml_dtypes.bfloat16)
            )


# --------------------------------------------------------------------------
# device kernel builder (shared by both layers)
# --------------------------------------------------------------------------

def build_layer_kernel(es: EdgeStruct, layer: int, with_bias: bool):
    """layer 1: tbl row [xh1(128)|e_src1(8)|junk], 256 bf16 elems,
               heads=8, csz=16, epilogue = softmax-div + ELU + transpose out.
       layer 2: tbl row [xh2(40)|e_src2(1)|junk], 128 bf16 elems, heads=1,
               csz=40, epilogue = softmax-div, row-major f32 out."""
    Npad, T_pad, G, S, ncols = es.Npad, es.T_pad, es.G, es.S, es.ncols
    n_groups, Qsz = es.n_groups, es.Qsz
    if layer == 1:
        ELEM, H, CSZ = 256, 8, 16
    else:
        ELEM, H, CSZ = 128, 1, 40
    # self-loop edges are not in the edge lists; their contribution is added
    # analytically in the tile epilogue from the core's own-node rows.
    MW = H * CSZ                      # message width (128 / 40)
    AW = MW + H                       # [msg | ea] width (136 / 41)
    WCOLS = AW                        # [W | W@a_src_blockdiag]
    WB = WCOLS + H                    # + W@a_dst_blockdiag
    GS = G * S
    NC4 = 4 * GS                      # == ncols

    nc = bacc.Bacc("TRN2", target_bir_lowering=False, debug=False,
                   num_devices=es.n_cores, num_swdge_queues=4)
    ap = {}
    ap["xT"] = nc.dram_tensor("xT", [P, Npad], BF16, kind="ExternalInput").ap()
    ap["xTm"] = nc.dram_tensor("xTm", [P, T_pad * P], BF16,
                               kind="ExternalInput").ap()
    ap["wboth"] = nc.dram_tensor("wboth", [P, WB], BF16,
                                 kind="ExternalInput").ap()
    ap["gidx"] = nc.dram_tensor("gidx", [n_groups * 4 * P, GS * 8], I16,
                                kind="ExternalInput").ap()
    ap["gdl"] = nc.dram_tensor("gdl", [n_groups * P, ncols], BF16,
                               kind="ExternalInput").ap()
    ap["iota_bf"] = nc.dram_tensor("iota_bf", [P, P], BF16,
                                   kind="ExternalInput").ap()
    ap["iota_rep"] = nc.dram_tensor("iota_rep", [P, 4 * S * P], BF16,
                                    kind="ExternalInput").ap()
    ap["idn_bf"] = nc.dram_tensor("idn_bf", [P, P], BF16,
                                  kind="ExternalInput").ap()
    if with_bias:
        ap["ones_bf"] = nc.dram_tensor("ones_bf", [1, P], BF16,
                                       kind="ExternalInput").ap()
        ap["brow"] = nc.dram_tensor("brow", [1, WB], BF16,
                                    kind="ExternalInput").ap()
    if layer == 1:
        out_ap = nc.dram_tensor("hT", [P, T_pad * P], BF16,
                                kind="ExternalOutput").ap()
    else:
        out_ap = nc.dram_tensor("logits", [T_pad * P, MW], F32,
                                kind="ExternalOutput").ap()
    # one table tensor per src quarter so quarter-q gathers only depend on
    # quarter-q prepass writes (overlaps gathers with the prepass tail)
    tblq = [nc.dram_tensor(f"tbl{q}", [Qsz, ELEM], BF16, kind="Internal").ap()
            for q in range(4)]
    own_tbl = nc.dram_tensor("own_tbl", [T_pad * P, WCOLS], BF16,
                             kind="Internal").ap()

    with tile.TileContext(nc) as tc, ExitStack() as ctx:
        cpool = ctx.enter_context(tc.tile_pool(name="consts", bufs=1))

        # ---- constants ----
        wboth = cpool.tile([P, WB], BF16)
        nc.sync.dma_start(wboth[:], ap["wboth"])
        iota_bf = cpool.tile([P, P], BF16)
        nc.sync.dma_start(iota_bf[:], ap["iota_bf"])
        iota_rep = cpool.tile([P, 4 * S * P], BF16)
        nc.sync.dma_start(iota_rep[:], ap["iota_rep"])
        idn_bf = cpool.tile([P, P], BF16)
        nc.sync.dma_start(idn_bf[:], ap["idn_bf"])
        if with_bias:
            ones_bf = cpool.tile([1, P], BF16)
            nc.sync.dma_start(ones_bf[:], ap["ones_bf"])
            brow = cpool.tile([1, WB], BF16)
            nc.sync.dma_start(brow[:], ap["brow"])
        edst_sb = cpool.tile([P, T_pad * H], BF16)

        with tc.tile_pool(name="pre_sb", bufs=4) as psb, \
                tc.tile_pool(name="pre_ps", bufs=4, space="PSUM") as pps:
            # ---- pre-pass A: full feature table [xh | e_src]; octets of
            # node tiles per load/store, pairs per PSUM bank, copies
            # alternating DVE/ACT to amortize per-instruction overheads ----
            NTq = Qsz // P
            for q in range(4):
                for i in range(0, NTq, 8):
                    w8 = min(8, NTq - i)
                    gt = q * NTq + i
                    xt8 = psb.tile([P, 8 * P], BF16, tag="xt")
                    nc.sync.dma_start(xt8[:, 0:w8 * P],
                                      ap["xT"][:, gt * P:(gt + w8) * P])
                    ot8 = psb.tile([P, 8 * WCOLS], BF16, tag="ot")
                    for j2 in range(w8 // 2):
                        ppt = pps.tile([P, 2 * WCOLS], F32, tag="ppt")
                        for jj in range(2):
                            j = 2 * j2 + jj
                            nc.tensor.matmul(
                                out=ppt[:, jj * WCOLS:(jj + 1) * WCOLS],
                                lhsT=xt8[:, j * P:(j + 1) * P],
                                rhs=wboth[:, 0:WCOLS],
                                start=True, stop=not with_bias,
                                skip_group_check=True)
                            if with_bias:
                                # bias row: feature columns get +bias (score
                                # columns of brow are zero); since sum(att)=1
                                # this reproduces "+ bias" after aggregation.
                                nc.tensor.matmul(
                                    out=ppt[:, jj * WCOLS:(jj + 1) * WCOLS],
                                    lhsT=ones_bf[:], rhs=brow[:, 0:WCOLS],
                                    start=False, stop=True,
                                    skip_group_check=True)
                        dst8 = ot8[:, j2 * 2 * WCOLS:(j2 + 1) * 2 * WCOLS]
                        if layer == 1 or j2 % 2 == 1:
                            # l1: all-ACT keeps DVE free (validated faster)
                            nc.scalar.activation(out=dst8, in_=ppt[:],
                                                 func=AF.Copy)
                        else:
                            nc.vector.tensor_copy(out=dst8, in_=ppt[:])
                    dst = tblq[q][i * P:(i + w8) * P, 0:WCOLS] \
                        .rearrange("(j p) w -> p j w", p=P)
                    nc.sync.dma_start(dst, ot8[:, 0:w8 * WCOLS].rearrange(
                        "p (j w) -> p j w", j=w8))

            # ---- pre-pass B: own rows [xh|e_src] (DRAM) and e_dst (SBUF),
            # pairs of tiles per load/psum/copy/store ----
            for t in range(0, T_pad, 2):
                xt2 = psb.tile([P, 2 * P], BF16, tag="xt2")
                nc.sync.dma_start(xt2[:], ap["xTm"][:, t * P:(t + 2) * P])
                po = pps.tile([P, 2 * WB], F32, tag="po")
                po3 = po[:].rearrange("p (j w) -> p j w", w=WB)
                for j in range(2):
                    nc.tensor.matmul(out=po[:, j * WB:(j + 1) * WB],
                                     lhsT=xt2[:, j * P:(j + 1) * P],
                                     rhs=wboth[:],
                                     start=True, stop=not with_bias,
                                     skip_group_check=True)
                    if with_bias:
                        nc.tensor.matmul(out=po[:, j * WB:(j + 1) * WB],
                                         lhsT=ones_bf[:], rhs=brow[:],
                                         start=False, stop=True,
                                         skip_group_check=True)
                oo = psb.tile([P, 2 * WCOLS], BF16, tag="oo")
                nc.vector.tensor_copy(
                    out=oo[:].rearrange("p (j w) -> p j w", j=2),
                    in_=po3[:, :, 0:WCOLS])
                nc.sync.dma_start(
                    own_tbl[t * P:(t + 2) * P, :]
                    .rearrange("(j p) w -> p j w", p=P),
                    oo[:].rearrange("p (j w) -> p j w", j=2))
                nc.scalar.activation(
                    out=edst_sb[:, t * H:(t + 2) * H]
                    .rearrange("p (j h) -> p j h", j=2),
                    in_=po3[:, :, WCOLS:WB], func=AF.Copy)

        # ---- edge pass ----
        sb = ctx.enter_context(tc.tile_pool(name="sb", bufs=2))
        gbp = ctx.enter_context(tc.tile_pool(name="gbuf", bufs=2))
        pacc = ctx.enter_context(tc.tile_pool(name="pacc", bufs=2,
                                              space="PSUM"))
        palp = ctx.enter_context(tc.tile_pool(name="palp", bufs=2,
                                              space="PSUM"))
        psdt = ctx.enter_context(tc.tile_pool(name="psdt", bufs=2,
                                              space="PSUM"))
        if layer == 1:
            ptp = ctx.enter_context(tc.tile_pool(name="ptp", bufs=2,
                                                 space="PSUM"))
        NSEG = 4 * S                  # chunks per tile
        for g in range(n_groups):
            # NOTE: the g=0 barrier is load-bearing — removing it (to
            # overlap quarter-0 gathers with the prepass tail) wedges the
            # device (NRT_EXEC_UNIT_UNRECOVERABLE).
            if g % BARRIER_EVERY == 0:
                tc.strict_bb_all_engine_barrier()
            gb = gbp.tile([P, ncols * ELEM], BF16, tag="gb")
            gb3 = gb[:].rearrange("p (c k) -> p c k", k=ELEM)
            idxs = sb.tile([P, 4 * GS * 8], I16, tag="idx")
            nc.sync.dma_start(
                idxs[:].rearrange("p (q j) -> p q j", q=4),
                ap["gidx"][g * 4 * P:(g + 1) * 4 * P, :]
                .rearrange("(q p) j -> p q j", p=P))
            dlt = sb.tile([P, ncols], BF16, tag="dl")
            nc.sync.dma_start(dlt[:], ap["gdl"][g * P:(g + 1) * P, :])
            own_g = sb.tile([P, G * WCOLS], BF16, tag="own")
            nc.sync.dma_start(
                own_g[:].rearrange("p (j w) -> p j w", j=G),
                own_tbl[g * G * P:(g + 1) * G * P, :]
                .rearrange("(j p) w -> p j w", p=P))
            if layer == 1:
                hT_g = sb.tile([P, G * P], BF16, tag="hTg")
            else:
                lg_g = sb.tile([P, G * MW], F32, tag="lgg")
            gq = 0
            for q in range(4):
                for c0 in range(0, GS, MAXGC):
                    c1 = min(c0 + MAXGC, GS)
                    nc.gpsimd.dma_gather(
                        out_ap=gb3[:, q * GS + c0:q * GS + c1, :],
                        in_ap=tblq[q],
                        idxs_ap=idxs[:, (q * GS + c0) * 8:(q * GS + c1) * 8],
                        num_idxs=(c1 - c0) * P,
                        num_idxs_reg=(c1 - c0) * P,
                        elem_size=ELEM,
                        queue_num=gq % 4,
                    )
                    gq += 1
            dlt4 = dlt[:].rearrange("p (q c) -> p q c", q=4)
            for t_loc in range(G):
                t = g * G + t_loc
                # ---- selection matrices: one is_equal builds all 4S seT
                # blocks of this tile; PE transposes give the sdT blocks ----
                seT = sb.tile([P, NSEG * P], BF16, tag="seT")
                seT4 = seT[:].rearrange("p (q s d) -> p q s d", q=4, d=P)
                nc.vector.tensor_tensor(
                    out=seT4,
                    in0=dlt4[:, :, t_loc * S:(t_loc + 1) * S]
                    .rearrange("p q (s o) -> p q s o", o=1)
                    .to_broadcast([P, 4, S, P]),
                    in1=iota_bf[:].rearrange("p (a b d) -> p a b d", a=1, b=1)
                    .to_broadcast([P, 4, S, P]),
                    op=ALU.is_equal)
                sdt = sb.tile([P, NSEG * P], BF16, tag="sdt")
                for b0 in range(0, NSEG, 4):
                    b1 = min(b0 + 4, NSEG)
                    ps = psdt.tile([P, 512], F32, tag="ps")
                    for j in range(b0, b1):
                        nc.tensor.matmul(
                            out=ps[:, (j - b0) * P:(j - b0 + 1) * P],
                            lhsT=seT[:, j * P:(j + 1) * P], rhs=idn_bf[:],
                            start=True, stop=True, skip_group_check=True)
                    # layer 1: all sdT copies on ACT (DVE is the bottleneck
                    # there); layer 2: alternate (ACT shift regressed l2)
                    if layer == 1 or (b0 // 4) % 2 == 0:
                        nc.scalar.activation(
                            out=sdt[:, b0 * P:b1 * P],
                            in_=ps[:, 0:(b1 - b0) * P], func=AF.Copy)
                    else:
                        nc.vector.tensor_copy(
                            out=sdt[:, b0 * P:b1 * P],
                            in_=ps[:, 0:(b1 - b0) * P])
                # ---- per-edge e_dst then alpha = lrelu(e_src + e_dst) ----
                pa = palp.tile([P, NSEG * H], F32, tag="pa")
                for j in range(NSEG):
                    nc.tensor.matmul(
                        out=pa[:, j * H:(j + 1) * H],
                        lhsT=sdt[:, j * P:(j + 1) * P],
                        rhs=edst_sb[:, t * H:(t + 1) * H],
                        start=True, stop=True, skip_group_check=True)
                albuf = sb.tile([P, NSEG * H], F32, tag="al")
                nc.vector.tensor_tensor(
                    out=albuf[:].rearrange("p (q s h) -> p q s h", q=4, h=H),
                    in0=pa[:].rearrange("p (q s h) -> p q s h", q=4, h=H),
                    in1=gb3[:, :, MW:MW + H]
                    .rearrange("p (q c) h -> p q c h", q=4)
                    [:, :, t_loc * S:(t_loc + 1) * S, :],
                    op=ALU.add)
                if USE_LRELU:
                    nc.scalar.activation(out=albuf[:], in_=albuf[:],
                                         func=AF.Lrelu, alpha=NEG_SLOPE)
                else:
                    al2 = sb.tile([P, NSEG * H], F32, tag="al2")
                    nc.vector.tensor_scalar_mul(out=al2[:], in0=albuf[:],
                                                scalar1=NEG_SLOPE)
                    nc.vector.tensor_tensor(out=albuf[:], in0=albuf[:],
                                            in1=al2[:], op=ALU.max)
                mea = sb.tile([P, NSEG * AW], BF16, tag="mea")
                mea3 = mea[:].rearrange("p (c w) -> p c w", w=AW)
                nc.scalar.activation(out=mea3[:, :, MW:MW + H],
                                     in_=albuf[:].rearrange(
                                         "p (c h) -> p c h", h=H),
                                     func=AF.Exp)
                # ---- messages msg = xh * ea (broadcast over channels) ----
                for q in range(4):
                    ea_q = mea3[:, q * S:(q + 1) * S, MW:MW + H] \
                        .rearrange("p s (h o) -> p s h o", o=1) \
                        .to_broadcast([P, S, H, CSZ])
                    xh_q = gb3[:, q * GS + t_loc * S:q * GS + (t_loc + 1) * S,
                               0:MW].rearrange("p s (h c) -> p s h c", c=CSZ)
                    msg_q = mea3[:, q * S:(q + 1) * S, 0:MW] \
                        .rearrange("p s (h c) -> p s h c", c=CSZ)
                    nc.vector.tensor_tensor(out=msg_q, in0=ea_q, in1=xh_q,
                                            op=ALU.mult)
                # ---- segment-reduce into the tile accumulator ----
                acc = pacc.tile([P, AW], F32, tag="acc")
                for j in range(NSEG):
                    nc.tensor.matmul(out=acc[:],
                                     lhsT=seT[:, j * P:(j + 1) * P],
                                     rhs=mea[:, j * AW:(j + 1) * AW],
                                     start=(j == 0), stop=(j == NSEG - 1),
                                     skip_group_check=True)
                # ---- tile epilogue (adds analytic self-loop term) ----
                own = own_g[:, t_loc * WCOLS:(t_loc + 1) * WCOLS]
                als = sb.tile([P, H], F32, tag="als")
                nc.vector.tensor_tensor(out=als[:], in0=own[:, MW:WCOLS],
                                        in1=edst_sb[:, t * H:(t + 1) * H],
                                        op=ALU.add)
                eas = sb.tile([P, H], F32, tag="eas")
                if USE_LRELU:
                    nc.scalar.activation(out=eas[:], in_=als[:], func=AF.Lrelu,
                                         alpha=NEG_SLOPE)
                else:
                    als2 = sb.tile([P, H], F32, tag="als2")
                    nc.vector.tensor_scalar_mul(out=als2[:], in0=als[:],
                                                scalar1=NEG_SLOPE)
                    nc.vector.tensor_tensor(out=eas[:], in0=als[:],
                                            in1=als2[:], op=ALU.max)
                nc.scalar.activation(out=eas[:], in_=eas[:], func=AF.Exp)
                # self message: own xh columns include +bias, matching tbl.
                smsg = sb.tile([P, MW], F32, tag="smsg")
                if H == 1:
                    nc.vector.tensor_tensor(
                        out=smsg[:], in0=eas[:, 0:1].to_broadcast([P, MW]),
                        in1=own[:, 0:MW], op=ALU.mult)
                else:
                    nc.vector.tensor_tensor(
                        out=smsg[:].rearrange("p (h c) -> p h c", c=CSZ),
                        in0=eas[:].rearrange("p (h o) -> p h o", o=1)
                        .to_broadcast([P, H, CSZ]),
                        in1=own[:, 0:MW].rearrange("p (h c) -> p h c", c=CSZ),
                        op=ALU.mult)
                unorm = sb.tile([P, MW], F32, tag="unorm")
                nc.vector.tensor_tensor(out=unorm[:], in0=acc[:, 0:MW],
                                        in1=smsg[:], op=ALU.add)
                den = sb.tile([P, H], F32, tag="den")
                # (+EPS dropped: den >= exp(lrelu(.)) > 0 always)
                nc.vector.tensor_tensor(out=den[:], in0=acc[:, MW:AW],
                                        in1=eas[:], op=ALU.add)
                rec = sb.tile([P, H], F32, tag="rec")
                nc.vector.reciprocal(out=rec[:], in_=den[:])
                if layer == 1:
                    otile = sb.tile([P, MW], F32, tag="otile")
                else:
                    otile = lg_g[:, t_loc * MW:(t_loc + 1) * MW]
                if H == 1:
                    nc.vector.tensor_tensor(
                        out=otile, in0=rec[:, 0:1].to_broadcast([P, MW]),
                        in1=unorm[:], op=ALU.mult)
                else:
                    rec3 = rec[:].rearrange("p (h o) -> p h o", o=1) \
                        .to_broadcast([P, H, CSZ])
                    acc3 = unorm[:].rearrange("p (h c) -> p h c", c=CSZ)
                    ot3 = otile[:].rearrange("p (h c) -> p h c", c=CSZ)
                    nc.vector.tensor_tensor(out=ot3, in0=rec3, in1=acc3,
                                            op=ALU.mult)
                if layer == 1:
                    # ELU: relu(x) + exp(min(x,0)) - 1, then transpose out
                    tmp = sb.tile([P, MW], F32, tag="tmp")
                    nc.vector.tensor_scalar_min(out=tmp[:], in0=otile[:],
                                                scalar1=0.0)
                    nc.scalar.activation(out=tmp[:], in_=tmp[:], func=AF.Exp)
                    nc.scalar.activation(out=otile[:], in_=otile[:],
                                         func=AF.Relu)
                    nc.vector.tensor_tensor(out=otile[:], in0=tmp[:],
                                            in1=otile[:], op=ALU.add)
                    obf = sb.tile([P, MW], BF16, tag="obf")
                    nc.scalar.activation(out=obf[:], in_=otile[:],
                                         func=AF.Copy, bias=-1.0)
                    tp = ptp.tile([P, P], F32, tag="tp")
                    nc.tensor.matmul(out=tp[:], lhsT=obf[:], rhs=idn_bf[:],
                                     start=True, stop=True,
                                     skip_group_check=True)
                    nc.scalar.activation(
                        out=hT_g[:, t_loc * P:(t_loc + 1) * P],
                        in_=tp[:], func=AF.Copy)
            if layer == 1:
                nc.sync.dma_start(out_ap[:, g * G * P:(g + 1) * G * P],
                                  hT_g[:])
            else:
                nc.sync.dma_start(
                    out_ap[g * G * P:(g + 1) * G * P, :]
                    .rearrange("(j p) w -> p j w", p=P),
                    lg_g[:].rearrange("p (j w) -> p j w", j=G))

    nc.compile()
    return nc


# --------------------------------------------------------------------------
# host orchestration
# --------------------------------------------------------------------------

def _consts_inputs(es):
    iota = np.arange(P, dtype=np.float32)
    iota_bf = np.tile(iota.astype(ml_dtypes.bfloat16)[None, :], (P, 1))
    return {
        "iota_bf": iota_bf,
        "iota_rep": np.ascontiguousarray(np.tile(iota_bf, (1, 4 * es.S))),
        "idn_bf": np.eye(P, dtype=ml_dtypes.bfloat16),
    }


def _blockdiag(att):
    """[H, C] attention vector -> [H*C, H] block-diagonal matrix."""
    H, C = att.shape
    out = np.zeros((H * C, H), np.float32)
    for h in range(H):
        out[h * C:(h + 1) * C, h] = att[h]
    return out


def _layer_inputs(es, xT_full, W, a_src, a_dst, b, n_cores, consts):
    """Build per-core input maps for one layer. xT_full: [P, Npad] bf16."""
    W = np.asarray(W, np.float32)
    wboth = np.concatenate(
        [W, W @ _blockdiag(np.asarray(a_src, np.float32)),
         W @ _blockdiag(np.asarray(a_dst, np.float32))],
        axis=1).astype(ml_dtypes.bfloat16)
    b = np.asarray(b, np.float32)
    with_bias = bool(np.any(b))
    T_pad, npc = es.T_pad, es.npc
    in_maps = []
    for c in range(n_cores):
        xTm = np.zeros((P, T_pad * P), ml_dtypes.bfloat16)
        xTm[:, :npc] = xT_full[:, c * npc:(c + 1) * npc]
        m = {
            "xT": xT_full, "xTm": xTm, "wboth": wboth,
            "gidx": es.gidx[c], "gdl": es.gdl[c],
            **consts,
        }
        if with_bias:
            brow = np.zeros((1, wboth.shape[1]), np.float32)
            brow[0, :len(b)] = b
            m["brow"] = brow.astype(ml_dtypes.bfloat16)
            m["ones_bf"] = np.ones((1, P), ml_dtypes.bfloat16)
        in_maps.append(m)
    return in_maps, with_bias


def run_gat(x, edge_index, W1, att_src1, att_dst1, b1, W2, att_src2, att_dst2,
            b2, N, n_cores, G=5, es=None, verbose=False):
    x = np.asarray(x, np.float32)
    src = np.asarray(edge_index[0]).astype(np.int64)
    dst = np.asarray(edge_index[1]).astype(np.int64)
    # self-loops are handled analytically inside the kernel epilogue

    if es is None:
        es = EdgeStruct(src, dst, N, n_cores, G=G)
    npc, Npad, T_pad = es.npc, es.Npad, es.T_pad

    consts = _consts_inputs(es)
    xT = np.zeros((P, Npad), ml_dtypes.bfloat16)
    xT[:, :N] = x.T.astype(ml_dtypes.bfloat16)

    in_maps, wb1 = _layer_inputs(es, xT, W1, att_src1, att_dst1, b1,
                                 n_cores, consts)
    nc1 = build_layer_kernel(es, 1, wb1)
    res1 = run_bass_kernel_spmd(nc1, in_maps, core_ids=list(range(n_cores)))
    hT = np.zeros((P, Npad), ml_dtypes.bfloat16)
    for c in range(n_cores):
        hT[:, c * npc:(c + 1) * npc] = res1.results[c]["hT"][:, :npc]

    in_maps2, wb2 = _layer_inputs(es, hT, W2, att_src2, att_dst2,
                                  np.zeros(1), n_cores, consts)
    nc2 = build_layer_kernel(es, 2, wb2)
    res2 = run_bass_kernel_spmd(nc2, in_maps2, core_ids=list(range(n_cores)))
    ncls = np.asarray(W2).shape[1]
    out = np.zeros((N, ncls), np.float32)
    for c in range(n_cores):
        out[c * npc:(c + 1) * npc] = res2.results[c]["logits"][:npc, :]
    out += np.asarray(b2, np.float32)[None, :]
    return out


def kernel(x, edge_index, W1, att_src1, att_dst1, b1, W2, att_src2, att_dst2,
           b2):
    N = int(np.asarray(x).shape[0])
    return run_gat(x, edge_index, W1, att_src1, att_dst1, b1, W2, att_src2,
                   att_dst2, b2, N=N, n_cores=8)
